# revision 12
# baseline (speedup 1.0000x reference)
"""Trainium2 Bass kernel for nn_LuongAttention.

Reference math (per batch b):
    S   = Dec @ Enc^T          # [T_dec, T_enc]
    Out = S @ Enc              # [T_dec, D]

By associativity:  Out = Dec @ (Enc^T @ Enc) = Dec @ G with G = Enc^T Enc
a [D, D] = [128, 128] Gram matrix.  This removes the [2048, 2048]
intermediate entirely (16x less FLOPs) and makes the kernel
memory-bound: ~1.5 MiB HBM I/O per core at fp16.

Sharding: data-parallel over batch B=8 -> one batch per NeuronCore.

Device-side layout trick: the host feeds Dec pre-transposed (DecT
[D, T]) and receives Out transposed (OutT [D, T]); the host transposes
the result back during the gather (pure layout permutation, no math).
With that:
  - G = sum_i EncTile_i^T @ EncTile_i  (accumulating PE matmuls, natural
    encoder layout - no transposes needed)
  - OutT = G @ DecT computed as matmul(lhsT=G, rhs=DecT chunk) with wide
    moving chunks (G is symmetric so lhsT=G gives G.T@X = G@X)
  - no PE transposes, no identity, minimal PSUM->SBUF copies

v2 schedule (from the v1 trace):
  - ALL DMA on the two HWDGE rings (sync/SP + scalar/ACT).  v1 put DecT
    on the SWDGE (gpsimd) queue, whose Q7 ring-drain kept every engine
    in the end-of-kernel barrier ~9 us after the last real work.
  - enc chunks are issued FIRST on both rings, dect behind them:
    HWDGE rings are FIFO and SDMA round-robins between rings packet-
    wise, so this lands the G-gating encoder at full HBM bandwidth
    instead of finishing together with everything else.
  - enc split in ENC_CHUNKS so Gram matmuls start when the first chunk
    lands, not when the full tensor does.
  - junk warm-up matmuls run during the load phase so the PE HAM clock
    gate (1.2 -> 2.4 GHz after ~3.4 us of activity) releases in time
    for the final matmuls.
  - final phase per 512-col chunk: PE matmul -> PSUM->SBUF cast copy
    (alternating DVE/ACT) -> store (alternating rings, queued after
    each ring's loads).
"""

import os
import sys
from contextlib import ExitStack

import numpy as np

for _p in (
    "/opt/trn_rl_repo",
    "/root/.axon_site",
    "/root/.axon_site/_ro/trn_rl_repo",
    "/root/.axon_site/_ro/pypackages",
):
    if os.path.isdir(_p) and _p not in sys.path:
        sys.path.append(_p)

import concourse.bacc as bacc
import concourse.mybir as mybir
import concourse.tile as tile
from concourse.bass_utils import run_bass_kernel_spmd

B, T, D, P = 8, 2048, 128, 128
NT = T // P  # 16 row tiles of 128

# tunables
MM_DTYPE = "fp16"  # "fp32" | "bf16" | "fp16"
FINAL_N = 512  # moving-operand width of the final matmul (1 PSUM bank)
STORE_N = 512  # store granularity
OUT_FP16 = True  # store OutT as fp16; host upcasts to fp32 after gather
WARMUP_MMS = 4  # junk matmuls issued early to trigger the PE HAM clock ramp
ENC_HEAD = 2  # tiles in the first (small) enc chunk per ring
DECT_ROW = T // 2 + 32  # padded dect row length (forces 2 KiB DMA packets)
# NOTE: same-lhsT N=512 accumulating warm-up matmuls do NOT pipeline —
# each costs ~730 ns on the PE queue (measured).  They must drain before
# the first Gram matmul, so keep count * 730ns < enc-chunk-0 landing time.


def _build_nc(mm_dtype=None):
    mm_dtype = mm_dtype or MM_DTYPE
    nc = bacc.Bacc("TRN2", target_bir_lowering=False, debug=False)
    f32 = mybir.dt.float32
    bf16 = mybir.dt.bfloat16
    fp16 = mybir.dt.float16

    in_dt = {"bf16": bf16, "fp16": fp16}.get(mm_dtype, f32)

    # enc arrives host-pre-shuffled to the SBUF layout [p, n*d] so chunk
    # loads are contiguous per partition.
    enc_h = nc.dram_tensor("enc", [P, NT * D], in_dt, kind="ExternalInput")
    # dect is host-padded to rows of DECT_ROW so its single SWDGE load has
    # 2 KiB descriptors instead of 4 KiB: the SDMA round-robin grants each
    # queue bandwidth proportional to packet size, and 4 KiB dect packets
    # starve the encoder stream that gates the Gram matrix (measured).
    dect_h = nc.dram_tensor("dect", [D, 2 * DECT_ROW], in_dt, kind="ExternalInput")
    out_dt = fp16 if OUT_FP16 else f32
    out_h = nc.dram_tensor("out", [D, T], out_dt, kind="ExternalOutput")

    # [p, n, d] view of encoder (p = row within tile, n = tile index)
    enc_v = enc_h.ap().rearrange("p (n d) -> p n d", d=D)
    dect_v = dect_h.ap().rearrange("p (c n) -> p c n", n=DECT_ROW)[:, :, : T // 2]
    out_v = out_h.ap()

    rings = [nc.sync, nc.scalar]

    with ExitStack() as ctx:
        tc = ctx.enter_context(tile.TileContext(nc))
        singles = ctx.enter_context(tc.tile_pool(name="singles", bufs=1))
        psum = ctx.enter_context(tc.tile_pool(name="psum", bufs=4, space="PSUM"))
        gpsum = ctx.enter_context(tc.tile_pool(name="gpsum", bufs=1, space="PSUM"))

        enc_sb = singles.tile([P, NT, D], in_dt)
        dect_sb = singles.tile([P, T], in_dt)
        out_sb = singles.tile([P, T], out_dt)

        # ---- loads ----
        # Three DMA queues run in parallel (HWDGE sync + HWDGE scalar +
        # SWDGE gpsimd) sharing the ~360 GB/s SDMA fabric.  DecT rides the
        # SWDGE queue as ONE instruction so the two HWDGE rings carry only
        # the encoder; within each ring a small first chunk lets the Gram
        # matmuls start ~1 us earlier (ring re-arm between instructions is
        # ~600 ns, paid only on the enc tail, not on dect).
        h = ENC_HEAD
        mid = h + (NT - 2 * h) // 2 + h  # = NT/2 + h
        enc_chunks = [
            (nc.sync, 0, h),
            (nc.scalar, h, 2 * h),
            (nc.sync, 2 * h, mid),
            (nc.scalar, mid, NT),
        ]
        for ring, lo, hi in enc_chunks:
            ring.dma_start(out=enc_sb[:, lo:hi, :], in_=enc_v[:, lo:hi, :])
        nc.gpsimd.dma_start(out=dect_sb[:], in_=dect_v[:])

        # ---- PE warm-up during the load phase ----
        if WARMUP_MMS:
            wsrc = singles.tile([P, 512], in_dt)
            nc.vector.memset(wsrc[:], 0.0)
            wps = gpsum.tile([P, 512], f32, tag="warm")
            for w in range(WARMUP_MMS):
                nc.tensor.matmul(
                    wps[:],
                    lhsT=wsrc[:, :P],
                    rhs=wsrc[:],
                    start=(w == 0),
                    stop=(w == WARMUP_MMS - 1),
                )

        # ---- Gram matrix construction (chunk-pipelined behind enc DMAs) ----
        g_sb = singles.tile([P, P], in_dt)
        g_ps = gpsum.tile([P, P], f32, tag="ga")
        for i in range(NT):
            nc.tensor.matmul(
                g_ps[:],
                lhsT=enc_sb[:, i, :],
                rhs=enc_sb[:, i, :],
                start=(i == 0),
                stop=(i == NT - 1),
            )
        nc.vector.tensor_copy(g_sb[:], g_ps[:])

        # ---- OutT = G @ DecT: wide moving chunks, stationary G ----
        # Per 512-col chunk: PE matmul -> (DVE|ACT) PSUM->SBUF cast copy.
        # Stores are two wide 1024-col DMAs (one per ring, behind that
        # ring's loads) to minimize ring re-arm bubbles.
        n_final = T // FINAL_N
        for c in range(n_final):
            op = psum.tile([P, FINAL_N], f32, tag="op")
            lo = c * FINAL_N
            nc.tensor.matmul(
                op[:],
                lhsT=g_sb[:],
                rhs=dect_sb[:, lo : lo + FINAL_N],
                start=True,
                stop=True,
            )
            if c % 2 == 0:
                nc.vector.tensor_copy(out_sb[:, lo : lo + FINAL_N], op[:])
            else:
                nc.scalar.copy(out_sb[:, lo : lo + FINAL_N], op[:])
            if (c + 1) * FINAL_N % STORE_N == 0:
                slo = (c + 1) * FINAL_N - STORE_N
                ring = nc.sync if (slo // STORE_N) % 2 == 0 else nc.scalar
                ring.dma_start(
                    out=out_v[:, slo : slo + STORE_N],
                    in_=out_sb[:, slo : slo + STORE_N],
                )

    nc.compile()
    return nc


_NC = {}


def _get_nc(mm_dtype=None):
    mm_dtype = mm_dtype or MM_DTYPE
    if mm_dtype not in _NC:
        _NC[mm_dtype] = _build_nc(mm_dtype)
    return _NC[mm_dtype]


def _np_in_dtype(mm_dtype):
    if mm_dtype == "bf16":
        import ml_dtypes

        return ml_dtypes.bfloat16
    if mm_dtype == "fp16":
        return np.float16
    return np.float32


def _run(enc, dec, mm_dtype=None, **kwargs):
    mm_dtype = mm_dtype or MM_DTYPE
    nc = _get_nc(mm_dtype)
    np_dt = _np_in_dtype(mm_dtype)
    in_maps = []
    hw = T // 2
    for b in range(B):
        dect = dec[b].T.astype(np_dt)  # [D, T]
        dect_p = np.zeros((D, 2 * DECT_ROW), np_dt)
        dect_p[:, :hw] = dect[:, :hw]
        dect_p[:, DECT_ROW : DECT_ROW + hw] = dect[:, hw:]
        in_maps.append(
            {
                "enc": np.ascontiguousarray(
                    enc[b].astype(np_dt).reshape(NT, P, D).transpose(1, 0, 2).reshape(P, NT * D)
                ),
                "dect": np.ascontiguousarray(dect_p),
            }
        )
    res = run_bass_kernel_spmd(nc, in_maps, core_ids=list(range(B)), **kwargs)
    out = np.stack([res.results[b]["out"].T.astype(np.float32) for b in range(B)], axis=0)
    return np.ascontiguousarray(out), res


def kernel(encoder_hidden_states, decoder_hidden_states):
    enc = np.ascontiguousarray(np.asarray(encoder_hidden_states, dtype=np.float32))
    dec = np.ascontiguousarray(np.asarray(decoder_hidden_states, dtype=np.float32))
    assert enc.shape == (B, T, D) and dec.shape == (B, T, D)
    out, _ = _run(enc, dec)
    return out


# revision 13
# speedup vs baseline: 1.2571x; 1.2571x over previous
"""Trainium2 Bass kernel for nn_LuongAttention.

Reference math (per batch b):
    S   = Dec @ Enc^T          # [T_dec, T_enc]
    Out = S @ Enc              # [T_dec, D]

By associativity:  Out = Dec @ (Enc^T @ Enc) = Dec @ G with G = Enc^T Enc
a [D, D] = [128, 128] Gram matrix.  This removes the [2048, 2048]
intermediate entirely (16x less FLOPs) and makes the kernel
memory-bound.

Sharding: data-parallel over batch B=8 -> one batch per NeuronCore.

Device-side layout trick: the host feeds Dec pre-transposed (DecT
[D, T]) and receives Out transposed (OutT [D, T]); the host transposes
the result back during the gather (pure layout permutation, no math).
With that:
  - G = sum_i EncTile_i^T @ EncTile_i  (accumulating PE matmuls, natural
    encoder layout - no transposes needed)
  - OutT = G @ DecT computed as matmul(lhsT=G, rhs=DecT chunk) with wide
    moving chunks (G is symmetric so lhsT=G gives G.T@X = G@X)
  - no PE transposes, no identity, minimal PSUM->PSUM copies

Load structure (measured to be the optimum of many variants): the two
encoder chunks ride the two HWDGE rings, the two DecT chunks ride the
SWDGE (gpsimd) queue, all issued immediately.  All streams use 2 KiB
(or 1 KiB for fp8 enc) per-partition runs; the SDMA round-robin grants
bandwidth proportional to packet size, so uniform chunk shapes keep the
encoder from being starved.

ENC_FP8: the encoder is loaded as float8_e4m3 and the Gram matrix is
accumulated from fp8 operands (fp32 PSUM).  Because G's diagonal grows
like T while the fp8 quantization noise grows like sqrt(T), the end-to-
end relative error stays ~1e-2 (host-verified 0.93e-2), under the 2e-2
gate, while cutting the encoder's HBM traffic in half.
"""

import os
import sys
from contextlib import ExitStack

import numpy as np

for _p in (
    "/opt/trn_rl_repo",
    "/root/.axon_site",
    "/root/.axon_site/_ro/trn_rl_repo",
    "/root/.axon_site/_ro/pypackages",
):
    if os.path.isdir(_p) and _p not in sys.path:
        sys.path.append(_p)

import concourse.bacc as bacc
import concourse.mybir as mybir
import concourse.tile as tile
from concourse.bass_utils import run_bass_kernel_spmd

B, T, D, P = 8, 2048, 128, 128
NT = T // P  # 16 row tiles of 128

# tunables
MM_DTYPE = "fp16"  # "fp16" | "fp8e" (fp8 encoder, fp16 decoder)
FINAL_N = 512  # moving-operand width of the final matmul (1 PSUM bank)
OUT_FP16 = True  # store OutT as fp16; host upcasts to fp32 after gather


def _build_nc(mm_dtype=None):
    mm_dtype = mm_dtype or MM_DTYPE
    nc = bacc.Bacc("TRN2", target_bir_lowering=False, debug=False)
    f32 = mybir.dt.float32
    fp16 = mybir.dt.float16
    fp8 = mybir.dt.float8e4

    enc_dt = fp8 if mm_dtype == "fp8e" else fp16
    dec_dt = fp16

    # enc arrives host-pre-shuffled to the SBUF layout [p, n*d] so chunk
    # loads are contiguous per partition.
    enc_h = nc.dram_tensor("enc", [P, NT * D], enc_dt, kind="ExternalInput")
    dect_h = nc.dram_tensor("dect", [D, T], dec_dt, kind="ExternalInput")
    out_dt = fp16 if OUT_FP16 else f32
    out_h = nc.dram_tensor("out", [D, T], out_dt, kind="ExternalOutput")

    # [p, n, d] view of encoder (p = row within tile, n = tile index)
    enc_v = enc_h.ap().rearrange("p (n d) -> p n d", d=D)
    dect_v = dect_h.ap()
    out_v = out_h.ap()

    with ExitStack() as ctx:
        tc = ctx.enter_context(tile.TileContext(nc))
        singles = ctx.enter_context(tc.tile_pool(name="singles", bufs=1))
        psum = ctx.enter_context(tc.tile_pool(name="psum", bufs=5, space="PSUM"))
        gpsum = ctx.enter_context(tc.tile_pool(name="gpsum", bufs=1, space="PSUM"))

        enc_sb = singles.tile([P, NT, D], enc_dt)
        dect_sb = singles.tile([P, T], dec_dt)
        out_sb = singles.tile([P, T], out_dt)

        # Interleave chunked loads across both HWDGE rings; DecT rides the
        # otherwise-idle SWDGE queue so all three queues stream in parallel.
        h = NT // 2
        nc.sync.dma_start(out=enc_sb[:, :h, :], in_=enc_v[:, :h, :])
        nc.scalar.dma_start(out=enc_sb[:, h:, :], in_=enc_v[:, h:, :])
        cs = T // 2
        for c in range(2):
            nc.gpsimd.dma_start(
                out=dect_sb[:, c * cs : (c + 1) * cs],
                in_=dect_v[:, c * cs : (c + 1) * cs],
            )

        # ---- Gram matrix construction ----
        g_sb = singles.tile([P, P], dec_dt)
        g_ps = gpsum.tile([P, P], f32, tag="ga")
        for i in range(NT):
            nc.tensor.matmul(
                g_ps[:],
                lhsT=enc_sb[:, i, :],
                rhs=enc_sb[:, i, :],
                start=(i == 0),
                stop=(i == NT - 1),
            )
        nc.vector.tensor_copy(g_sb[:], g_ps[:])

        # ---- OutT = G @ DecT: wide moving chunks, stationary G ----
        # Pipeline: PE matmul -> (DVE|ACT) PSUM->SBUF copy -> store.
        n_final = T // FINAL_N
        for c in range(n_final):
            op = psum.tile([P, FINAL_N], f32, tag="op")
            lo = c * FINAL_N
            nc.tensor.matmul(
                op[:],
                lhsT=g_sb[:],
                rhs=dect_sb[:, lo : lo + FINAL_N],
                start=True,
                stop=True,
            )
            if c % 2 == 0:
                nc.vector.tensor_copy(out_sb[:, lo : lo + FINAL_N], op[:])
            else:
                nc.scalar.copy(out_sb[:, lo : lo + FINAL_N], op[:])
            ring = nc.sync if c % 2 == 0 else nc.scalar
            ring.dma_start(
                out=out_v[:, lo : lo + FINAL_N],
                in_=out_sb[:, lo : lo + FINAL_N],
            )

    nc.compile()
    return nc


_NC = {}


def _get_nc(mm_dtype=None):
    mm_dtype = mm_dtype or MM_DTYPE
    if mm_dtype not in _NC:
        _NC[mm_dtype] = _build_nc(mm_dtype)
    return _NC[mm_dtype]


def _np_dtypes(mm_dtype):
    import ml_dtypes

    enc_dt = ml_dtypes.float8_e4m3 if mm_dtype == "fp8e" else np.float16
    return enc_dt, np.float16


def _run(enc, dec, mm_dtype=None, **kwargs):
    mm_dtype = mm_dtype or MM_DTYPE
    nc = _get_nc(mm_dtype)
    enc_np, dec_np = _np_dtypes(mm_dtype)
    in_maps = []
    for b in range(B):
        in_maps.append(
            {
                "enc": np.ascontiguousarray(
                    enc[b].astype(enc_np).reshape(NT, P, D).transpose(1, 0, 2).reshape(P, NT * D)
                ),
                "dect": np.ascontiguousarray(dec[b].T.astype(dec_np)),
            }
        )
    res = run_bass_kernel_spmd(nc, in_maps, core_ids=list(range(B)), **kwargs)
    out = np.stack([res.results[b]["out"].T.astype(np.float32) for b in range(B)], axis=0)
    return np.ascontiguousarray(out), res


def kernel(encoder_hidden_states, decoder_hidden_states):
    enc = np.ascontiguousarray(np.asarray(encoder_hidden_states, dtype=np.float32))
    dec = np.ascontiguousarray(np.asarray(decoder_hidden_states, dtype=np.float32))
    assert enc.shape == (B, T, D) and dec.shape == (B, T, D)
    out, _ = _run(enc, dec)
    return out
